# revision 1
# baseline (speedup 1.0000x reference)
"""Trainium2 Bass kernel for nn_AttentionLayer (GN -> conv1x1 -> self-attn ->
cross-attn -> conv1x1, residuals). Data-parallel over batch: 16 samples split
across 8 NeuronCores (2 samples/core), no collectives.

v2: fp8e4 DoubleRow matmuls (2x PE throughput) on the big contractions:
SA q/k/v projections, SA sim, SA attn@v + softmax column sums, CA k/v
projections (from fp8 ctx^T) and the CA out-projection. conv_in/conv_out
and the whole small-K cross-attention core (sim K=64, attn@v K=77) stay
bf16 -- fp8 buys nothing there and bf16 protects the error budget.

Scale folding: fp8 weights are host-scaled by power-of-2 s (~32); the
inverses ride for free: SA exp scale absorbs 1/(swq*swk), the softmax
column-sum selectors (ones2 / emat) carry the v-scale so the reciprocal
normalization absorbs 1/swv, and 1/sco lands in the h2b epilogue scalar.
All PSUM->SBUF epilogues are therefore pure casts spread across
Pool/DVE/ACT so no single engine gates the PE.

One activation table (natural_log_exp_and_others, explicitly preloaded):
GroupNorm rsqrt is exp(-0.5*ln(var+eps)); softmax reciprocals are DVE.
h0 is bf16 and pre-doubled (GN is scale-invariant) so the self-attn
double residual is a plain add. ca_bo is folded into b_out on the host
(b_out' = b_out + w_out @ ca_bo). Softmax normalizations multiply
straight out of PSUM (broadcast matmuls land in spare PSUM banks).
"""

import sys

if "/opt/trn_rl_repo" not in sys.path:
    sys.path.insert(0, "/opt/trn_rl_repo")

import contextlib

import numpy as np
import ml_dtypes

import concourse.bass as bass
import concourse.mybir as mybir
from concourse import bacc
import concourse.tile as tile
from concourse.bass import ts
from concourse.bass_utils import run_bass_kernel_spmd
from concourse.masks import make_identity

BF = mybir.dt.bfloat16
F8 = mybir.dt.float8e4
F32 = mybir.dt.float32
AF = mybir.ActivationFunctionType
ALU = mybir.AluOpType
AX = mybir.AxisListType
DR = mybir.MatmulPerfMode.DoubleRow
E4 = ml_dtypes.float8_e4m3

NCORES = 8
BS = 2            # samples per core
CIN = 256         # input channels
INNER = 512       # inner channels
HW = 1024         # 32*32 spatial
CTXN = 77
CTXD = 768
HEADS = 8
DH = 64
EPS = 1e-5
SCALE_SA = float(INNER) ** -0.5   # self-attn scale (c = 512)
SCALE_CA = float(DH) ** -0.5      # cross-attn scale (1/8)

NT_CIN = CIN // 128    # 2 partition tiles of input channels
NT_IN = INNER // 128   # 4 partition tiles of inner channels
NT_HW = HW // 128      # 8 spatial tiles
NT_D = CTXD // 128     # 6 partition tiles of context dim
NH = HW // 512         # 2 free halves of spatial

ACT_TABLE_ID = 6       # natural_log_exp_and_others (serves exp/ln/square/copy)

PHASE_MARKS = []       # (label, instruction-id) build-time phase boundaries
SCHEDULE = [("A", 0), ("A", 1), ("Ep", 0), ("Ep", 1), ("Bs", 0), ("Bq", 0),
            ("Bs", 1), ("C", 0), ("Bq", 1), ("Eq", 0), ("C", 1), ("F", 0),
            ("Eq", 1), ("IJ", 0), ("F", 1), ("IJ", 1)]


def _gn_sums(nc, small, sqp, x_sb, nt, tag, cts=None):
    """Per-channel sum and sum-of-squares for the given ct tiles."""
    s12 = small.tile([128, nt, 2], F32, tag=f"{tag}_s12")
    for ct in (range(nt) if cts is None else cts):
        sq = sqp.tile([128, 1024], BF, tag="sq_scratch",
                      name=f"{tag}_sq{ct}")
        nc.vector.tensor_reduce(out=s12[:, ct, 0:1], in_=x_sb[:, ct, :],
                                axis=AX.X, op=ALU.add)
        nc.scalar.activation(out=sq[:], in_=x_sb[:, ct, :], func=AF.Square,
                             accum_out=s12[:, ct, 1:2])
    return s12


def _gn_finish(nc, psB, small, s12, nt, gmat_sb, gexp_sb, gam_sb,
               bet_sb, inv_n, tag, eps_ap):
    """Group stats -> per-channel (scale, negbias)."""
    psg = psB.tile([32, 2], F32, tag="psB")
    for ct in range(nt):
        nc.tensor.matmul(psg[:], lhsT=gmat_sb[:, ct, :], rhs=s12[:, ct, :],
                         start=(ct == 0), stop=(ct == nt - 1))
    # mm cols: 0=-mu, 1=ex2, 2=rsig, 3=scratch
    mm = small.tile([32, 4], F32, tag=f"{tag}_mm")
    nc.vector.tensor_scalar(out=mm[:, 0:2], in0=psg[:], scalar1=inv_n,
                            scalar2=None, op0=ALU.mult)
    # col3 = mu*mu - ex2 = -var
    nc.vector.scalar_tensor_tensor(out=mm[:, 3:4], in0=mm[:, 0:1],
                                   scalar=mm[:, 0:1], in1=mm[:, 1:2],
                                   op0=ALU.mult, op1=ALU.subtract)
    # rsig = exp(-0.5 * ln(var + eps));  ln/exp live in the one act table
    nc.scalar.activation(out=mm[:, 3:4], in_=mm[:, 3:4], func=AF.Ln,
                         bias=eps_ap[:32, :], scale=-1.0)
    nc.scalar.activation(out=mm[:, 2:3], in_=mm[:, 3:4], func=AF.Exp,
                         scale=-0.5)
    scb = small.tile([128, nt, 3], F32, tag=f"{tag}_scb")
    for ct in range(nt):
        pse = psB.tile([128, 2], F32, tag="psB")
        nc.tensor.matmul(pse[:], lhsT=gexp_sb[:, ct, :], rhs=mm[:, 0:3:2],
                         start=True, stop=True)
        nc.vector.tensor_tensor(out=scb[:, ct, 0:1], in0=pse[:, 1:2],
                                in1=gam_sb[:, ct:ct + 1], op=ALU.mult)
        # col1 = mu*scale - beta (consumers apply x*scale - col1);
        # col2 = -col1 for ACT Identity consumers (scale*x + bias form)
        nc.vector.scalar_tensor_tensor(out=scb[:, ct, 1:2], in0=pse[:, 0:1],
                                       scalar=scb[:, ct, 0:1],
                                       in1=bet_sb[:, ct:ct + 1],
                                       op0=ALU.mult, op1=ALU.subtract)
        nc.vector.tensor_scalar(out=scb[:, ct, 2:3], in0=scb[:, ct, 1:2],
                                scalar1=-1.0, scalar2=None, op0=ALU.mult)
    return scb


def build(scales, repeat=1):
    swq, swk, swv, sck, scv, sco = scales
    exp_sa = SCALE_SA / (swq * swk)   # q,k carry their weight scales
    exp_ca = SCALE_CA / sck           # kT carries sck (qT is bf16/natural)
    inv_co = 1.0 / sco
    nc = bacc.Bacc("TRN2", target_bir_lowering=False, debug=False)
    d = {}

    def di(name, shape, dt):
        d[name] = nc.dram_tensor(name, shape, dt, kind="ExternalInput").ap()

    di("x", [BS, 128, NT_CIN, HW], BF)           # host pre-tiled channel-major
    di("ctx", [BS, CTXN, CTXD], BF)
    # GN1 path first so its DMAs land before the big weights
    di("g1mat", [128, NT_CIN, 32], F32)
    di("g1exp", [32, NT_CIN, 128], F32)
    di("gn1_g", [128, NT_CIN], F32)
    di("gn1_b", [128, NT_CIN], F32)
    di("ca_wk_T", [128, NT_D, INNER], F8)
    di("ca_wv_T", [128, NT_D, INNER], F8)
    di("w_in_T", [128, NT_CIN, INNER], BF)
    di("b_in2", [128, NT_IN], F32)               # 2*b_in (h0 is pre-doubled)
    di("g2mat", [128, NT_IN, 32], F32)
    di("g2exp", [32, NT_IN, 128], F32)
    di("sa_gn_g", [128, NT_IN], F32)
    di("sa_gn_b", [128, NT_IN], F32)
    di("sa_wq_T", [128, NT_IN, INNER], F8)
    di("sa_wk_T", [128, NT_IN, INNER], F8)
    di("sa_wv_T", [128, NT_IN, INNER], F8)  # holds (sa_wp @ sa_wv).T
    di("ca_wq_T", [128, NT_IN, INNER], BF)
    di("ca_wo_T", [128, NT_IN, INNER], F8)
    di("w_out_T", [128, NT_IN, CIN], BF)
    di("b_out2", [128, NT_CIN], F32)             # b_out + w_out @ ca_bo
    di("emat", [CTXN, 2, 128], F8)               # head-pair selector * scv
    out_d = nc.dram_tensor("out", [BS, CIN, HW], F32, kind="ExternalOutput").ap()

    with tile.TileContext(nc) as tc:
        with contextlib.ExitStack() as ctx:
            singles = ctx.enter_context(tc.tile_pool(name="singles", bufs=1))
            xpool = ctx.enter_context(tc.tile_pool(name="xpool", bufs=3))
            h0pool = ctx.enter_context(tc.tile_pool(name="h0pool", bufs=3))
            big8 = ctx.enter_context(tc.tile_pool(name="big8", bufs=7))
            big16 = ctx.enter_context(tc.tile_pool(name="big16", bufs=6))
            attnp = ctx.enter_context(tc.tile_pool(name="attnp", bufs=2))
            tmpp = ctx.enter_context(tc.tile_pool(name="tmpp", bufs=2))
            small = ctx.enter_context(tc.tile_pool(name="small", bufs=3))
            sqp = ctx.enter_context(tc.tile_pool(name="sqp", bufs=2))
            crossp = ctx.enter_context(tc.tile_pool(name="crossp", bufs=2))
            expp = ctx.enter_context(tc.tile_pool(name="expp", bufs=3))
            outp = ctx.enter_context(tc.tile_pool(name="outp", bufs=2))
            recp = ctx.enter_context(tc.tile_pool(name="recp", bufs=2))
            # PSUM (8 banks): psA = 3 x 2-bank rings, psB = 2 x 1-bank
            # (softmax column sums live pinned in psB halves during the
            # attention loops; broadcast tiles borrow psA slots)
            psA = ctx.enter_context(tc.tile_pool(name="psA", bufs=3, space="PSUM"))
            psB = ctx.enter_context(tc.tile_pool(name="psB", bufs=2, space="PSUM"))

            # ---- x loads first: their transfers must win the DMA engines
            # over the weight queue (first data the kernel needs) ----
            x_tiles = {}
            for s in range(BS):
                x_sb = xpool.tile([128, NT_CIN, HW], BF, tag="x",
                                  name=f"x_sb{s}")
                xeng = nc.sync if s == 0 else nc.gpsimd
                for ct in range(NT_CIN):
                    xeng.dma_start(out=x_sb[:, ct, :], in_=d["x"][s, :, ct])
                x_tiles[s] = x_sb

            # ---- load weights & constants once ----
            def wload(name, shape, dt):
                t = singles.tile(shape, dt, tag=name)
                nc.sync.dma_start(out=t[:], in_=d[name])
                return t

            g1mat = wload("g1mat", [128, NT_CIN, 32], F32)
            g1exp = wload("g1exp", [32, NT_CIN, 128], F32)
            gn1_g = wload("gn1_g", [128, NT_CIN], F32)
            gn1_b = wload("gn1_b", [128, NT_CIN], F32)
            w_in = wload("w_in_T", [128, NT_CIN, INNER], BF)
            b_in2 = wload("b_in2", [128, NT_IN], F32)
            g2mat = wload("g2mat", [128, NT_IN, 32], F32)
            g2exp = wload("g2exp", [32, NT_IN, 128], F32)
            gn2_g = wload("sa_gn_g", [128, NT_IN], F32)
            gn2_b = wload("sa_gn_b", [128, NT_IN], F32)
            wq = wload("sa_wq_T", [128, NT_IN, INNER], F8)
            wk = wload("sa_wk_T", [128, NT_IN, INNER], F8)
            cwk = wload("ca_wk_T", [128, NT_D, INNER], F8)
            cwv = wload("ca_wv_T", [128, NT_D, INNER], F8)
            wv = wload("sa_wv_T", [128, NT_IN, INNER], F8)
            cwq = wload("ca_wq_T", [128, NT_IN, INNER], BF)
            cwo = wload("ca_wo_T", [128, NT_IN, INNER], F8)
            w_out = wload("w_out_T", [128, NT_IN, CIN], BF)
            b_out2 = wload("b_out2", [128, NT_CIN], F32)
            emat = wload("emat", [CTXN, 2, 128], F8)
            id_bf = singles.tile([128, 128], BF, tag="id_bf")
            make_identity(nc, id_bf[:])
            eps_t = singles.tile([128, 1], F32, tag="eps")
            nc.gpsimd.memset(eps_t[:], EPS)
            ones2 = singles.tile([128, 2, 128], F8, tag="ones2")
            nc.gpsimd.memset(ones2[:], float(swv))
            # pin the single activation table
            nc.scalar.add_instruction(mybir.InstLoadActFuncSet(
                name=nc.get_next_instruction_name(), ins=[], outs=[],
                act_func_set_id=ACT_TABLE_ID))

            def gen_A(s, st):
                # GN1 + conv_in (bf16; fp8 here hurts the error budget).
                # x arrives in two halves so stats start at half-arrival;
                # GN2 sums run per-m right behind the conv epilogue.
                x_sb = x_tiles[s]
                s12 = _gn_sums(nc, small, sqp, x_sb, NT_CIN, f"gn1_{s}")
                scb = _gn_finish(nc, psB, small, s12, NT_CIN, g1mat, g1exp,
                                 gn1_g, gn1_b, 1.0 / (8 * HW), f"gn1_{s}", eps_t)
                gn1 = big16.tile([128, NT_CIN, HW], BF, tag="big16")
                yield
                for ct in range(NT_CIN):
                    nc.vector.tensor_scalar(out=gn1[:, ct, :], in0=x_sb[:, ct, :],
                                            scalar1=scb[:, ct, 0:1],
                                            scalar2=scb[:, ct, 1:2],
                                            op0=ALU.mult, op1=ALU.subtract)
                # h0 = 2*(conv_in(gn1) + b_in)  (pre-doubled residual master;
                # GN2 is scale-invariant so stats/apply need no adjustment)
                h0 = h0pool.tile([128, NT_IN, HW], BF, tag="h0")
                s12b = small.tile([128, NT_IN, 2], F32, tag=f"gn2_{s}_s12")
                for m in range(NT_IN):
                    ps = psA.tile([128, HW], F32, tag="psA")
                    for h in range(NH):
                        for c in range(NT_CIN):
                            nc.tensor.matmul(ps[:, ts(h, 512)],
                                             lhsT=w_in[:, c, ts(m, 128)],
                                             rhs=gn1[:, c, ts(h, 512)],
                                             start=(c == 0), stop=(c == NT_CIN - 1))
                    # accum_out gives the GN2 per-channel sum for free
                    nc.scalar.activation(out=h0[:, m, :], in_=ps[:],
                                         func=AF.Identity,
                                         bias=b_in2[:, m:m + 1], scale=2.0,
                                         accum_out=s12b[:, m, 0:1])
                    yield
                st["x_sb"], st["h0"], st["s12b"] = x_sb, h0, s12b

            def gen_Bstats(s, st):
                s12b = st["s12b"]
                for ct in range(NT_IN):
                    sq = sqp.tile([128, 1024], BF, tag="sq_scratch",
                                  name=f"sq{s}_{ct}")
                    if ct % 2 == 0:
                        nc.scalar.activation(out=sq[:], in_=st["h0"][:, ct, :],
                                             func=AF.Square,
                                             accum_out=s12b[:, ct, 1:2])
                    else:
                        nc.vector.tensor_tensor(out=sq[:],
                                                in0=st["h0"][:, ct, :],
                                                in1=st["h0"][:, ct, :],
                                                op=ALU.mult)
                        nc.vector.tensor_reduce(out=s12b[:, ct, 1:2],
                                                in_=sq[:], axis=AX.X,
                                                op=ALU.add)
                st["scb2"] = _gn_finish(nc, psB, small, s12b, NT_IN, g2mat,
                                        g2exp, gn2_g, gn2_b, 1.0 / (16 * HW),
                                        f"gn2_{s}", eps_t)
                yield

            def gen_Bqkv(s, st):
                gn2 = big8.tile([128, NT_IN, HW], F8, tag="big8")
                scb2 = st["scb2"]
                with nc.allow_low_precision(reason="fp8 matmul inputs"):
                    for ct in range(NT_IN):
                        nc.vector.tensor_scalar(out=gn2[:, ct, :],
                                                in0=st["h0"][:, ct, :],
                                                scalar1=scb2[:, ct, 0:1],
                                                scalar2=scb2[:, ct, 1:2],
                                                op0=ALU.mult, op1=ALU.subtract)
                    yield
                    q_sb = big8.tile([128, NT_IN, HW], F8, tag="big8")
                    k_sb = big8.tile([128, NT_IN, HW], F8, tag="big8")
                    for dst, w, ceng in ((q_sb, wq, nc.scalar),
                                         (k_sb, wk, nc.vector)):
                        for m in range(NT_IN):
                            ps = psA.tile([128, HW], F32, tag="psA")
                            for h in range(NH):
                                for p in range(NT_IN // 2):
                                    nc.tensor.matmul(
                                        ps[:, ts(h, 512)],
                                        lhsT=w[:, 2 * p:2 * p + 2, ts(m, 128)],
                                        rhs=gn2[:, 2 * p:2 * p + 2, ts(h, 512)],
                                        start=(p == 0), stop=(p == 1),
                                        perf_mode=DR)
                            nc.scalar.copy(out=dst[:, m, ts(0, 512)],
                                           in_=ps[:, ts(0, 512)])
                            nc.vector.tensor_copy(out=dst[:, m, ts(1, 512)],
                                                  in_=ps[:, ts(1, 512)])
                            yield
                st["q"], st["k"], st["gn2"] = q_sb, k_sb, gn2

            def gen_C(s, st):
                # transposed self-attention: simT = k^T q; softmax along
                # partitions: column sums via fp8-DR ones-matmul (ones carry
                # swv), reciprocal on DVE, broadcast into a spare PSUM tile,
                # normalization multiplies straight out of PSUM.
                q_sb, k_sb, gn2, h0 = st["q"], st["k"], st["gn2"], st["h0"]
                vT = big8.tile([128, NT_HW, 512], F8, tag="big8")
                h1b = big16.tile([128, NT_IN, HW], BF, tag="big16")
                eTall = attnp.tile([128, NT_HW, HW], F8, tag="eTall")
                ssh = [psB.tile([128, 512], F32, tag="psB", name=f"ssh{s}_{h}")
                       for h in range(NH)]
                with nc.allow_low_precision(reason="fp8 attn"):
                    for jb in range(NT_HW):
                        ps_sim = psA.tile([128, HW], F32, tag="psA")
                        for h in range(NH):
                            for p in range(NT_IN // 2):
                                nc.tensor.matmul(
                                    ps_sim[:, ts(h, 512)],
                                    lhsT=k_sb[:, 2 * p:2 * p + 2, ts(jb, 128)],
                                    rhs=q_sb[:, 2 * p:2 * p + 2, ts(h, 512)],
                                    start=(p == 0), stop=(p == 1), perf_mode=DR)
                        nc.scalar.activation(out=eTall[:, jb, :], in_=ps_sim[:],
                                             func=AF.Exp, scale=exp_sa)
                        if jb % 2 == 1:
                            for h in range(NH):
                                nc.tensor.matmul(
                                    ssh[h][:],
                                    lhsT=ones2[:],
                                    rhs=eTall[:, jb - 1:jb + 1, ts(h, 512)],
                                    start=(jb == 1), stop=(jb == NT_HW - 1),
                                    perf_mode=DR)
                        yield
                        if jb < NT_HW // 2:
                            jp = jb
                            psv = psA.tile([128, HW], F32, tag="psA")
                            for jj in range(2):
                                jbv = 2 * jp + jj
                                for p in range(NT_IN // 2):
                                    nc.tensor.matmul(
                                        psv[:, ts(jj, 512)],
                                        lhsT=gn2[:, 2 * p:2 * p + 2, ts(jbv, 128)],
                                        rhs=wv[:, 2 * p:2 * p + 2, :],
                                        start=(p == 0), stop=(p == 1),
                                        perf_mode=DR)
                            nc.scalar.copy(out=vT[:, 2 * jp, :],
                                           in_=psv[:, ts(0, 512)])
                            nc.vector.tensor_copy(out=vT[:, 2 * jp + 1, :],
                                                  in_=psv[:, ts(1, 512)])
                            yield
                    recB = crossp.tile([128, HW], BF, tag="rB")
                    for h in range(NH):
                        nc.vector.reciprocal(out=recB[:, ts(h, 512)],
                                             in_=ssh[h][:])
                    yield
                    for c2 in range(NT_IN):
                        ps_o = psA.tile([128, HW], F32, tag="psA")
                        for h in range(NH):
                            for p in range(NT_HW // 2):
                                nc.tensor.matmul(
                                    ps_o[:, ts(h, 512)],
                                    lhsT=vT[:, 2 * p:2 * p + 2, ts(c2, 128)],
                                    rhs=eTall[:, 2 * p:2 * p + 2, ts(h, 512)],
                                    start=(p == 0), stop=(p == NT_HW // 2 - 1),
                                    perf_mode=DR)
                        tmp = tmpp.tile([128, HW], BF, tag="tmp")
                        nc.vector.tensor_tensor(out=tmp[:], in0=ps_o[:],
                                                in1=recB[:], op=ALU.mult)
                        # h1 = h0(pre-doubled) + proj
                        nc.vector.tensor_tensor(out=h1b[:, c2, :],
                                                in0=h0[:, c2, :], in1=tmp[:],
                                                op=ALU.add)
                        yield
                st["h1b"] = h1b

            def gen_Epre(s, st):
                # x-independent cross-attn K/V path: runs at the very start
                # to fill the PE while GroupNorm stats hold everything else
                ctx_sb = crossp.tile([CTXN, CTXD], BF, tag="ctx")
                nc.gpsimd.dma_start(out=ctx_sb[:], in_=d["ctx"][s])
                # pad per-block stride to 80 so bf16 PSUM offsets stay aligned
                psT3 = psB.tile([128, NT_D, 80], BF, tag="psB")
                for dd in range(NT_D):
                    nc.tensor.transpose(psT3[:, dd, :CTXN], ctx_sb[:, ts(dd, 128)],
                                        id_bf[:CTXN, :CTXN])
                ctxT = crossp.tile([128, NT_D, CTXN], F8, tag="ctxT")
                yield
                with nc.allow_low_precision(reason="fp8 ctx"):
                    nc.vector.tensor_copy(out=ctxT[:], in_=psT3[:, :, :CTXN])
                    # kT [512, 77] = sck * k^T  (sck folded into exp_ca)
                    ps_kt = psB.tile([128, NT_IN, CTXN], F32, tag="psB")
                    for m in range(NT_IN):
                        for p in range(NT_D // 2):
                            nc.tensor.matmul(
                                ps_kt[:, m, :],
                                lhsT=cwk[:, 2 * p:2 * p + 2, ts(m, 128)],
                                rhs=ctxT[:, 2 * p:2 * p + 2, :],
                                start=(p == 0), stop=(p == NT_D // 2 - 1),
                                perf_mode=DR)
                    kT = crossp.tile([128, NT_IN, CTXN], F8, tag="kT")
                    nc.vector.tensor_copy(out=kT[:], in_=ps_kt[:])
                    yield
                    # v [77, 512] = scv * v  (scv folded via emat -> rec)
                    ps_v = psB.tile([CTXN, 512], F32, tag="psB")
                    for dd in range(NT_D):
                        nc.tensor.matmul(ps_v[:],
                                         lhsT=ctxT[:, dd, :],
                                         rhs=cwv[:, dd, :],
                                         start=(dd == 0), stop=(dd == NT_D - 1))
                    v_sb = crossp.tile([CTXN, 512], F8, tag="v_sb")
                    nc.vector.tensor_copy(out=v_sb[:], in_=ps_v[:])
                st["kT"], st["v"] = kT, v_sb

            def gen_Eq(s, st):
                # qT [512, 1024] fp8, straight off the bf16 residual master
                h1b = st["h1b"]
                qT = big8.tile([128, NT_IN, HW], F8, tag="big8")
                with nc.allow_low_precision(reason="fp8 attn"):
                    for m in range(NT_IN):
                        ps = psA.tile([128, HW], F32, tag="psA")
                        for h in range(NH):
                            for c in range(NT_IN):
                                nc.tensor.matmul(ps[:, ts(h, 512)],
                                                 lhsT=cwq[:, c, ts(m, 128)],
                                                 rhs=h1b[:, c, ts(h, 512)],
                                                 start=(c == 0),
                                                 stop=(c == NT_IN - 1))
                        nc.scalar.copy(out=qT[:, m, ts(0, 512)],
                                       in_=ps[:, ts(0, 512)])
                        nc.vector.tensor_copy(out=qT[:, m, ts(1, 512)],
                                              in_=ps[:, ts(1, 512)])
                        yield
                st["qT"] = qT

            def gen_F(s, st):
                # transposed cross-attention in fp8.  Per head-pair (= one
                # 128-channel block): sim + exp, then a full-ones DoubleRow
                # matmul yields the softmax column sums already broadcast to
                # all 128 partitions; a [128,512] reciprocal makes the
                # normalizer, attn@v lands beside it, one multiply finishes.
                kT, v_sb, qT = st["kT"], st["v"], st["qT"]
                ox8 = big8.tile([128, NT_IN, HW], F8, tag="big8")
                with nc.allow_low_precision(reason="fp8 attn"):
                    for ct in range(NT_IN):
                        eTp = expp.tile([CTXN, 2, HW], F8, tag="expT")
                        for hh in range(2):
                            hd = 2 * ct + hh
                            po = (hd % 2) * 64
                            mt = hd // 2
                            ps_sT = psA.tile([CTXN, HW], F32, tag="psA")
                            for h in range(NH):
                                nc.tensor.matmul(ps_sT[:, ts(h, 512)],
                                                 lhsT=kT[po:po + 64, mt, :],
                                                 rhs=qT[po:po + 64, mt, ts(h, 512)],
                                                 start=True, stop=True)
                            nc.scalar.activation(out=eTp[:, hh, :], in_=ps_sT[:],
                                                 func=AF.Exp, scale=exp_ca)
                        rB = crossp.tile([128, HW], BF, tag="rB")
                        for h in range(NH):
                            hsB = psB.tile([128, 512], F32, tag="psB")
                            nc.tensor.matmul(hsB[:], lhsT=emat[:],
                                             rhs=eTp[:, :, ts(h, 512)],
                                             start=True, stop=True,
                                             perf_mode=DR)
                            nc.vector.reciprocal(out=rB[:, ts(h, 512)],
                                                 in_=hsB[:])
                        yield
                        ps_or = psA.tile([128, HW], F32, tag="psA")
                        for hh in range(2):
                            hd = 2 * ct + hh
                            for h in range(NH):
                                nc.tensor.matmul(
                                    ps_or[hh * 64:hh * 64 + 64, ts(h, 512)],
                                    lhsT=v_sb[:, ts(hd, DH)],
                                    rhs=eTp[:, hh, ts(h, 512)],
                                    start=True, stop=True)
                        nc.vector.tensor_tensor(out=ox8[:, ct, :],
                                                in0=ps_or[:],
                                                in1=rB[:], op=ALU.mult)
                        yield
                st["ox8"] = ox8

            def gen_IJo(s, st):
                ox8, h1b = st["ox8"], st["h1b"]
                h2b = big16.tile([128, NT_IN, HW], BF, tag="big16")
                for m in range(NT_IN):
                    ps = psA.tile([128, HW], F32, tag="psA")
                    for h in range(NH):
                        for p in range(NT_IN // 2):
                            nc.tensor.matmul(
                                ps[:, ts(h, 512)],
                                lhsT=cwo[:, 2 * p:2 * p + 2, ts(m, 128)],
                                rhs=ox8[:, 2 * p:2 * p + 2, ts(h, 512)],
                                start=(p == 0), stop=(p == 1), perf_mode=DR)
                    # h2 = ps/sco + h1  (ca_bo folded into b_out2)
                    nc.vector.scalar_tensor_tensor(out=h2b[:, m, :], in0=ps[:],
                                                   scalar=inv_co,
                                                   in1=h1b[:, m, :],
                                                   op0=ALU.mult, op1=ALU.add)
                    yield
                st["h2b"] = h2b

            def gen_IJc(s, st):
                h2b, x_sb = st["h2b"], st["x_sb"]
                for m in range(NT_CIN):
                    ps = psA.tile([128, HW], F32, tag="psA")
                    for h in range(NH):
                        for c in range(NT_IN):
                            nc.tensor.matmul(ps[:, ts(h, 512)],
                                             lhsT=w_out[:, c, ts(m, 128)],
                                             rhs=h2b[:, c, ts(h, 512)],
                                             start=(c == 0), stop=(c == NT_IN - 1))
                    ot = outp.tile([128, HW], F32, tag="outt")
                    for h in range(NH):
                        nc.vector.scalar_tensor_tensor(
                            out=ot[:, ts(h, 512)], in0=ps[:, ts(h, 512)],
                            scalar=b_out2[:, m:m + 1],
                            in1=x_sb[:, m, ts(h, 512)],
                            op0=ALU.add, op1=ALU.add)
                        nc.sync.dma_start(out=out_d[s, ts(m, 128), ts(h, 512)],
                                          in_=ot[:, ts(h, 512)])
                    yield

            # Fine-grained interleave: concurrent phase pairs are zipped at
            # psA-allocation granularity so the shared PSUM ring alternates
            # between them (trace order = ring order) instead of
            # serializing whole phases.
            def _mark(label):
                nm = nc.get_next_instruction_name()
                PHASE_MARKS.append((label, int(nm.split("-")[1])))

            def run(*gens, ratio=None):
                live = [iter(g) for g in gens]
                weights = list(ratio) if ratio else [1] * len(gens)
                while live:
                    for gi, g in enumerate(list(live)):
                        for _ in range(weights[gi] if gi < len(weights) else 1):
                            try:
                                next(g)
                            except StopIteration:
                                live.remove(g)
                                break

            lp = ctx.enter_context(
                nc.allow_low_precision(reason="fp8/bf16 kernel"))
            tail = None
            for _ in range(repeat):
                st = [dict(), dict()]
                _mark("A01")
                gens = [gen_A(0, st[0]), gen_A(1, st[1]),
                        gen_Epre(0, st[0]), gen_Epre(1, st[1])]
                if tail is not None:
                    gens.append(tail)
                run(*gens)
                _mark("Bs0")
                run(gen_Bstats(0, st[0]))
                _mark("Bq0")
                run(gen_Bqkv(0, st[0]), gen_Bstats(1, st[1]))
                _mark("C0")
                run(gen_C(0, st[0]), gen_Bqkv(1, st[1]))
                _mark("Eq0")
                run(gen_C(1, st[1]), gen_Eq(0, st[0]), ratio=(4, 1))
                _mark("F0")
                run(gen_F(0, st[0]), gen_Eq(1, st[1]), ratio=(3, 1))
                _mark("IJ0")
                run(gen_F(1, st[1]), gen_IJo(0, st[0]), ratio=(2, 1))
                _mark("IJ1")
                run(gen_IJc(0, st[0]), gen_IJo(1, st[1]))
                tail = gen_IJc(1, st[1])
            run(tail)
            _mark("end")

    nc.compile()
    return nc


# ---------------------------------------------------------------------------
# host-side wrapper
# ---------------------------------------------------------------------------

def _tile_rows(a, dt):
    """[R, M] -> [128, R//128, M] partition-tiled, contiguous."""
    r, m = a.shape
    return np.ascontiguousarray(
        a.reshape(r // 128, 128, m).transpose(1, 0, 2).astype(dt))


def _col_tiled(v, dt=np.float32):
    """[C] -> [128, C//128]."""
    c = v.shape[0]
    return np.ascontiguousarray(v.reshape(c // 128, 128).T.astype(dt))


def _pow2_scale(w):
    """Power-of-2 scale <= 32 keeping max|w|*s well under fp8e4's 240."""
    m = float(np.max(np.abs(w)))
    if m == 0:
        return 1.0
    return float(min(32.0, 2.0 ** np.floor(np.log2(96.0 / m))))


def prep_inputs(inputs):
    bf = ml_dtypes.bfloat16
    f32 = np.float32
    x = np.asarray(inputs["x"], f32).reshape(NCORES, BS, CIN, HW)
    # [core, s, 256, 1024] -> [core, s, 128, 2, 1024], bf16 on the host so
    # the two samples' loads ride different (non-casting) DMA rings
    x = np.ascontiguousarray(
        x.reshape(NCORES, BS, NT_CIN, 128, HW).transpose(0, 1, 3, 2, 4)
        .astype(bf))
    ctxa = np.asarray(inputs["context"], f32).astype(bf).reshape(
        NCORES, BS, CTXN, CTXD)

    g1mat = np.zeros((CIN, 32), f32)
    g1mat[np.arange(CIN), np.arange(CIN) // 8] = 1.0
    g2mat = np.zeros((INNER, 32), f32)
    g2mat[np.arange(INNER), np.arange(INNER) // 16] = 1.0

    wpv = np.asarray(inputs["sa_wp"], f32) @ np.asarray(inputs["sa_wv"], f32)
    fpw = {
        "sa_wq_T": np.asarray(inputs["sa_wq"], f32).T,
        "sa_wk_T": np.asarray(inputs["sa_wk"], f32).T,
        "sa_wv_T": wpv.T,
        "ca_wk_T": np.asarray(inputs["ca_wk"], f32).T,
        "ca_wv_T": np.asarray(inputs["ca_wv"], f32).T,
        "ca_wo_T": np.asarray(inputs["ca_wo"], f32).T,
    }
    keys = ("sa_wq_T", "sa_wk_T", "sa_wv_T", "ca_wk_T", "ca_wv_T", "ca_wo_T")
    scales = tuple(_pow2_scale(fpw[k]) for k in keys)
    scv = scales[4]
    emat = np.zeros((CTXN, 2, 128), f32)
    emat[:, 0, 0:64] = scv
    emat[:, 1, 64:128] = scv
    b_out2 = (np.asarray(inputs["b_out"], f32) +
              np.asarray(inputs["w_out"], f32) @ np.asarray(inputs["ca_bo"], f32))

    com = {
        "w_in_T": _tile_rows(np.asarray(inputs["w_in"], f32).T, bf),
        "ca_wq_T": _tile_rows(np.asarray(inputs["ca_wq"], f32).T, bf),
        "w_out_T": _tile_rows(np.asarray(inputs["w_out"], f32).T, bf),
        "b_in2": _col_tiled(2.0 * np.asarray(inputs["b_in"], f32)),
        "b_out2": _col_tiled(b_out2),
        "gn1_g": _col_tiled(np.asarray(inputs["gn1_g"], f32)),
        "gn1_b": _col_tiled(np.asarray(inputs["gn1_b"], f32)),
        "sa_gn_g": _col_tiled(np.asarray(inputs["sa_gn_g"], f32)),
        "sa_gn_b": _col_tiled(np.asarray(inputs["sa_gn_b"], f32)),
        "g1mat": _tile_rows(g1mat, f32),
        "g1exp": np.ascontiguousarray(
            g1mat.T.reshape(32, NT_CIN, 128).astype(f32)),
        "g2mat": _tile_rows(g2mat, f32),
        "g2exp": np.ascontiguousarray(
            g2mat.T.reshape(32, NT_IN, 128).astype(f32)),
        "emat": emat.astype(E4),
    }
    for k, s in zip(keys, scales):
        com[k] = _tile_rows(fpw[k] * s, E4)
    return scales, [{**com, "x": np.ascontiguousarray(x[c]),
                     "ctx": np.ascontiguousarray(ctxa[c])}
                    for c in range(NCORES)]


def assemble_output(results):
    # results: list (per core) of {"out": [BS, 256, 1024]}
    outs = np.stack([r["out"] for r in results])      # [8, 2, 256, 1024]
    return outs.reshape(16, CIN, 32, 32)


_CACHE = {}


def kernel(**inputs) -> np.ndarray:
    scales, in_maps = prep_inputs(inputs)
    if _CACHE.get("scales") != scales:
        _CACHE["nc"] = build(scales, repeat=1)
        _CACHE["scales"] = scales
    nc = _CACHE["nc"]
    res = run_bass_kernel_spmd(nc, in_maps, core_ids=list(range(NCORES)))
    return assemble_output(res.results)


# ---------------------------------------------------------------------------
# device-resident runner (for timing): keeps inputs on device, feeds outputs
# back as donated output buffers so repeated calls ship no data.
# ---------------------------------------------------------------------------

class DeviceRunner:
    def __init__(self, nc):
        import jax
        from jax.sharding import Mesh, PartitionSpec, NamedSharding
        from jax.experimental.shard_map import shard_map
        from concourse.bass2jax import (_bass_exec_p, install_neuronx_cc_hook,
                                        partition_id_tensor)
        install_neuronx_cc_hook()
        self.jax = jax
        self.nc = nc
        pname = nc.partition_id_tensor.name if nc.partition_id_tensor else None
        in_names, out_names, out_avals, zero_outs = [], [], [], []
        for alloc in nc.m.functions[0].allocations:
            if not isinstance(alloc, mybir.MemoryLocationSet):
                continue
            name = alloc.memorylocations[0].name
            if alloc.kind == "ExternalInput":
                if name != pname:
                    in_names.append(name)
            elif alloc.kind == "ExternalOutput":
                out_names.append(name)
                shape = tuple(alloc.tensor_shape)
                dtype = mybir.dt.np(alloc.dtype)
                out_avals.append(jax.core.ShapedArray(shape, dtype))
                zero_outs.append(np.zeros(shape, dtype))
        self.in_names, self.out_names = in_names, out_names
        self.out_avals, self.zero_outs = out_avals, zero_outs
        n_params, n_outs = len(in_names), len(out_avals)
        names_all = in_names + out_names + ([pname] if pname else [])

        def _body(*args):
            operands = list(args)
            if pname is not None:
                operands.append(partition_id_tensor())
            return tuple(_bass_exec_p.bind(
                *operands, out_avals=tuple(out_avals),
                in_names=tuple(names_all), out_names=tuple(out_names),
                lowering_input_output_aliases=(), sim_require_finite=True,
                sim_require_nnan=True, nc=nc))

        devices = jax.devices()[:NCORES]
        self.mesh = Mesh(np.asarray(devices), ("core",))
        self.sh = NamedSharding(self.mesh, PartitionSpec("core"))
        self.fn = jax.jit(
            shard_map(_body, mesh=self.mesh,
                      in_specs=(PartitionSpec("core"),) * (n_params + n_outs),
                      out_specs=(PartitionSpec("core"),) * n_outs,
                      check_rep=False),
            donate_argnums=tuple(range(n_params, n_params + n_outs)),
            keep_unused=True)

    def put_inputs(self, in_maps):
        jax = self.jax
        concat = [np.concatenate([np.asarray(m[n]) for m in in_maps], axis=0)
                  for n in self.in_names]
        self.in_dev = [jax.device_put(a, self.sh) for a in concat]
        self.outs = self.fn(*self.in_dev, *[
            jax.device_put(np.zeros((NCORES * z.shape[0], *z.shape[1:]), z.dtype),
                           self.sh) for z in self.zero_outs])
        jax.block_until_ready(self.outs)

    def run_once(self):
        self.outs = self.fn(*self.in_dev, *self.outs)
        return self.outs

    def time_iters(self, iters):
        import time as _t
        jax = self.jax
        t0 = _t.perf_counter()
        for _ in range(iters):
            self.outs = self.fn(*self.in_dev, *self.outs)
        jax.block_until_ready(self.outs)
        return (_t.perf_counter() - t0) / iters

    def get_outputs(self):
        res = [np.asarray(o) for o in self.jax.block_until_ready(self.outs)]
        per_core = []
        for c in range(NCORES):
            m = {}
            for i, nme in enumerate(self.out_names):
                shp = self.out_avals[i].shape
                m[nme] = res[i].reshape(NCORES, *shp)[c]
            per_core.append(m)
        return per_core

